# revision 26
# baseline (speedup 1.0000x reference)
"""Trainium2 Bass kernel for masked multi-head attention block (qkv proj +
softmax(QK^T/sqrt(hd)) with boolean mask + AV + output proj).

Sharding (8 cores): core c -> batch b=c//2, head-half hh=c%2 (8 of 16 heads).
Tensor parallelism over heads: each core projects q/k/v only for its 8 heads
(column-sharded w_qkv), runs attention for those heads over the full 2048x2048
sequence, and computes a partial output projection (row-sharded w_proj). The
two partial y's per batch are summed on the host during unsharding (+ b_proj).
No device collectives; no redundant k/v compute.

On-chip layout is feature-major: T(x)=[cin, seq], q/k=[head dims, seq].
S^T tiles [k_seq, q_seq] come from lhsT=k slices, rhs=q slices, two heads
packed into the 128-row PE array via tile_position row groups; the two heads'
PSUM banks are adjacent so ONE activation instruction exps both ([128,1024]).
Softmax runs without max subtraction (logits are O(3); exp cannot overflow).
The keep-mask (~mask, bf16) is applied multiplicatively after exp on the DVE
in its 2x 16-bit mode, broadcast across the two heads with a stride-0 AP.
AV uses lhsT=[V_h | ones] so PSUM row 64 accumulates softmax denominators;
reciprocal via the fast approx DVE op, broadcast across head dims by a tiny
f32r matmul. Mask tiles are streamed per q-chunk to fit SBUF.
"""

from contextlib import ExitStack

import numpy as np

import concourse.bass as bass
import concourse.tile as tile
from concourse import bacc, mybir

F32 = mybir.dt.float32
BF16 = mybir.dt.bfloat16
F32R = mybir.dt.float32r
FP8 = mybir.dt.float8e4
DR = mybir.MatmulPerfMode.DoubleRow
SW = 64.0   # fp8 weight pre-scale (w std 0.02 is subnormal in e4m3)
SA = 64.0   # fp8 ao pre-scale
Exp = mybir.ActivationFunctionType.Exp
Identity = mybir.ActivationFunctionType.Identity

P = 128


class Dims:
    def __init__(self, S=2048, C=1024, H=16, HD=64, NCORE=8):
        self.S, self.C, self.H, self.HD = S, C, H, HD
        self.HPC = H // 2            # heads per core (head-half sharding)
        self.NPAIR = self.HPC // 2   # head pairs per core
        self.CT = C // P             # cin tiles
        self.KT = S // P             # k seq tiles
        self.QCW = 512               # q chunk width
        self.QC = S // self.QCW      # q chunks
        self.CH = self.HPC * HD      # channels per core (= 512)
        self.FT = self.CH // P       # feature tiles for out proj
        self.OCW = 512
        self.OC = C // self.OCW
        self.ST = S // P             # seq tiles


FULL = Dims()


def r(ap):
    return ap.bitcast(F32R)


def emit_body(ctx, tc, d, io, rep=1):
    nc = tc.nc
    HD = d.HD
    xT_h, wqkT_h, wvT_h, wpT_h, bqk_h, bv_h, maskT_h, y_h = io
    ctx.enter_context(nc.allow_low_precision(
        reason="bf16 matmul pipeline; accumulation stays fp32 in PSUM"))

    const = ctx.enter_context(tc.tile_pool(name="const", bufs=1))
    ones_f32 = const.tile([P, max(d.HD, d.KT)], F32)
    nc.vector.memset(ones_f32[:], 1.0)
    ones_row = const.tile([P, max(d.HD, d.KT)], F32R)
    nc.vector.tensor_copy(ones_row[:], ones_f32[:])
    ones_bf = const.tile([P, max(d.HD, d.KT)], BF16)
    nc.vector.tensor_copy(ones_bf[:], ones_f32[:])
    # biases: bqk_sb[:, j] = bqk[j*128 : (j+1)*128]  (4 q cols then 4 k cols)
    bqk_sb = const.tile([P, 2 * d.CH // P], F32)
    nc.sync.dma_start(out=bqk_sb[:], in_=bqk_h[:].rearrange("(j p) -> p j", p=P))
    bias_pool = ctx.enter_context(tc.tile_pool(name="bias_pool", bufs=1))
    bv_ap = bv_h[:]
    bv_bcast = bias_pool.tile([P, d.CH], F32, tag="bias")
    nc.sync.dma_start(
        out=bv_bcast[:],
        in_=bass.AP(tensor=bv_ap.tensor, offset=bv_ap.offset, ap=[[0, P]] + list(bv_ap.ap)),
    )

    ao_pool = ctx.enter_context(tc.tile_pool(name="ao_pool", bufs=1))
    ao = ao_pool.tile([P, d.NPAIR, d.S], BF16)
    # all 8 PSUM banks: psS ring of 3 x 2-bank tiles (proj/bc/psy share the
    # ring, using half a tile) + 2 single-bank AV accumulators
    psS = ctx.enter_context(tc.tile_pool(name="psS", bufs=3, space="PSUM"))
    psAV = ctx.enter_context(tc.tile_pool(name="psAV", bufs=2, space="PSUM"))

    mask_pool = ctx.enter_context(tc.tile_pool(name="mask_pool", bufs=2))
    xt_pool = ctx.enter_context(tc.tile_pool(name="xt_pool", bufs=1))
    wqk_pool = ctx.enter_context(tc.tile_pool(name="wqk_pool", bufs=2))
    wv_pool = ctx.enter_context(tc.tile_pool(name="wv_pool", bufs=1))
    qk_pool = ctx.enter_context(tc.tile_pool(name="qk_pool", bufs=1))
    v_pool = ctx.enter_context(tc.tile_pool(name="v_pool", bufs=1))
    p_pool = ctx.enter_context(tc.tile_pool(name="p_pool", bufs=8))
    small_pool = ctx.enter_context(tc.tile_pool(name="small_pool", bufs=2))
    wp_pool = ctx.enter_context(tc.tile_pool(name="wp_pool", bufs=1))
    y_pool = ctx.enter_context(tc.tile_pool(name="y_pool", bufs=2))

    # inputs staged once, reused by every rep
    xt = xt_pool.tile([P, d.CT, d.S], BF16)
    xT_r = xT_h[:].rearrange("(t p) s -> p t s", p=P)
    for ct in range(d.CT):
        nc.sync.dma_start(out=xt[:, ct, :], in_=xT_r[:, ct, :])

    wqkT_r = wqkT_h[:].rearrange("(t p) c -> p t c", p=P)
    wvT_r = wvT_h[:].rearrange("(t p) c -> p t c", p=P)
    wpT_r = wpT_h[:].rearrange("(t p) c -> p t c", p=P)
    maskT_r = maskT_h[:].rearrange("(t p) q -> p t q", p=P)

    # weights staged once
    wv_c = wv_pool.tile([P, d.CT, d.CH], BF16)
    nc.sync.dma_start(out=wv_c[:], in_=wvT_r[:])
    wp_sb = wp_pool.tile([P, d.FT, d.C], BF16)
    nc.sync.dma_start(out=wp_sb[:], in_=wpT_r[:])

    # v with interleaved ones columns: v_sb[:, kt, h*(HD+1)+HD] = 1
    v_sb = v_pool.tile([P, d.KT, d.HPC * (HD + 1)], BF16)
    for h in range(d.HPC):
        cc = h * (HD + 1) + HD
        nc.vector.tensor_copy(
            v_sb[:, :, cc:cc + 1].rearrange("p t x -> p (t x)"),
            ones_bf[:, 0:d.KT])

    q_all = qk_pool.tile([P, d.NPAIR, d.S], BF16, name="q_all", tag="q")
    k_all = qk_pool.tile([P, d.NPAIR, d.S], BF16, name="k_all", tag="k")

    LAG = 3  # kt iterations the AV matmuls trail the S matmuls by

    for rep_i in range(rep):

        def gen_vproj():
            for st in range(d.ST):
                psv = psS.tile([P, 2, d.QCW], F32, tag="s2", name="psv")[:, 0, :]
                for ct in range(d.CT):
                    nc.tensor.matmul(
                        psv, xt[:, ct, st * P:(st + 1) * P], wv_c[:, ct, :],
                        start=(ct == 0), stop=(ct == d.CT - 1))
                    if ct % 4 == 3:
                        yield
                dst = v_sb[:, st, :].rearrange("p (h x) -> p h x", x=HD + 1)[:, :, 0:HD]
                nc.vector.tensor_tensor(
                    dst, psv.rearrange("p (h x) -> p h x", x=HD),
                    bv_bcast[:].rearrange("p (h x) -> p h x", x=HD),
                    mybir.AluOpType.add)
                yield

        def gen_qkproj(pair):
            wq_p = wqk_pool.tile([P, d.CT, P], BF16, name="wq_p")
            nc.sync.dma_start(out=wq_p[:], in_=wqkT_r[:, :, pair * P:(pair + 1) * P])
            wk_p = wqk_pool.tile([P, d.CT, P], BF16, name="wk_p")
            nc.sync.dma_start(
                out=wk_p[:], in_=wqkT_r[:, :, d.CH + pair * P:d.CH + (pair + 1) * P])
            yield
            for qc in range(d.QC):
                psq = psS.tile([P, 2, d.QCW], F32, tag="s2", name="psq")[:, 0, :]
                for ct in range(d.CT):
                    nc.tensor.matmul(
                        psq, wq_p[:, ct, :],
                        xt[:, ct, qc * d.QCW:(qc + 1) * d.QCW],
                        start=(ct == 0), stop=(ct == d.CT - 1))
                    if ct % 4 == 3:
                        yield
                nc.scalar.activation(
                    q_all[:, pair, qc * d.QCW:(qc + 1) * d.QCW], psq,
                    Identity, bias=bqk_sb[:, pair:pair + 1])
            for kc in range(d.QC):
                psk = psS.tile([P, 2, d.QCW], F32, tag="s2", name="psk")[:, 0, :]
                for ct in range(d.CT):
                    nc.tensor.matmul(
                        psk, wk_p[:, ct, :],
                        xt[:, ct, kc * d.QCW:(kc + 1) * d.QCW],
                        start=(ct == 0), stop=(ct == d.CT - 1))
                    if ct % 4 == 3:
                        yield
                nc.scalar.activation(
                    k_all[:, pair, kc * d.QCW:(kc + 1) * d.QCW], psk,
                    Identity, bias=bqk_sb[:, d.NPAIR + pair:d.NPAIR + pair + 1])

        proj_done = {}

        def gen_allproj():
            yield from gen_vproj()
            for pair in range(d.NPAIR):
                yield from gen_qkproj(pair)
                proj_done[pair] = True
            yield

        gproj = gen_allproj()

        def drain_proj(pair):
            while not proj_done.get(pair):
                next(gproj, None)

        # prologue: v proj + pair 0 q/k fully (needed before attention starts)
        drain_proj(0)

        def gen_outproj(qc):
            """Output projection for q-chunk qc's tokens (ao columns final)."""
            q0 = qc * d.QCW
            for st4 in range(d.QCW // P):
                st = (q0 // P) + st4
                for oc in range(d.OC):
                    psy = psS.tile([P, 2, d.QCW], F32, tag="s2", name="psy")[:, 0, :]
                    for ft in range(d.FT):
                        nc.tensor.matmul(
                            psy, ao[:, ft, st * P:(st + 1) * P],
                            wp_sb[:, ft, oc * d.OCW:(oc + 1) * d.OCW],
                            start=(ft == 0), stop=(ft == d.FT - 1))
                        if ft % 2 == 1:
                            yield
                    y_sb = y_pool.tile([P, d.OCW], F32, tag="y")
                    nc.vector.tensor_copy(y_sb[:], psy)
                    nc.sync.dma_start(
                        out=y_h[st * P:(st + 1) * P, oc * d.OCW:(oc + 1) * d.OCW],
                        in_=y_sb[:])

        gout = None
        mask_tiles = {}

        def load_mask(qc):
            m = mask_pool.tile([P, d.KT, d.QCW], BF16, name="m_qc")
            nc.sync.dma_start(
                out=m[:], in_=maskT_r[:, :, qc * d.QCW:(qc + 1) * d.QCW])
            mask_tiles[qc] = m

        load_mask(0)
        for qc in range(d.QC):
            if qc + 1 < d.QC:
                load_mask(qc + 1)
            mq = mask_tiles.pop(qc)
            q0 = qc * d.QCW
            for pair in range(d.NPAIR):
                drain_proj(pair)
                av = [psAV.tile([HD + 1, d.QCW], F32, tag="av", name=f"av{_h}")
                      for _h in range(2)]
                pend = []

                def flush_av(n):
                    for _ in range(n):
                        kt_, p_t = pend.pop(0)
                        for h01 in range(2):
                            vh = pair * 2 + h01
                            nc.tensor.matmul(
                                av[h01][:],
                                v_sb[:, kt_, vh * (HD + 1):(vh + 1) * (HD + 1)],
                                p_t[:, h01, :],
                                start=(kt_ == 0), stop=(kt_ == d.KT - 1))

                for kt in range(d.KT):
                    s2 = psS.tile([P, 2, d.QCW], F32, tag="s2", name="s2")
                    for h01 in range(2):
                        nc.tensor.matmul(
                            s2[:, h01, :],
                            k_all[h01 * HD:(h01 + 1) * HD, pair, kt * P:(kt + 1) * P],
                            q_all[h01 * HD:(h01 + 1) * HD, pair, q0:q0 + d.QCW],
                            start=True, stop=True, tile_position=(h01 * HD, 0))
                    p_t = p_pool.tile([P, 2, d.QCW], BF16, tag="p", name="p_t")
                    nc.scalar.activation(p_t[:], s2[:], Exp)
                    m_ap = mq[:, kt, :]
                    m_bcast = bass.AP(
                        tensor=m_ap.tensor, offset=m_ap.offset,
                        ap=[list(m_ap.ap[0])] + [[0, 2]] + [list(m_ap.ap[1])])
                    nc.vector.tensor_tensor(p_t[:], p_t[:], m_bcast,
                                            mybir.AluOpType.mult)
                    pend.append((kt, p_t))
                    if len(pend) > LAG:
                        flush_av(1)
                    # step projection / trailing out-proj generators
                    next(gproj, None)
                    if gout is not None and kt % 2 == 0:
                        next(gout, None)
                flush_av(len(pend))
                for h01 in range(2):
                    recip = small_pool.tile([1, d.QCW], F32R, tag="recip")
                    nc.vector.reciprocal(recip[:], av[h01][HD:HD + 1, :])
                    bc_ps = psS.tile([P, 2, d.QCW], F32, tag="s2", name="bc_ps")[0:HD, 0, :]
                    nc.tensor.matmul(bc_ps, ones_row[0:1, 0:HD], recip[:],
                                     start=True, stop=True)
                    bc_sb = small_pool.tile([HD, d.QCW], F32, tag="bc")
                    nc.vector.tensor_copy(bc_sb[:], bc_ps)
                    nc.vector.tensor_tensor(
                        ao[h01 * HD:(h01 + 1) * HD, pair, q0:q0 + d.QCW],
                        av[h01][0:HD, :], bc_sb[:], mybir.AluOpType.mult)
            # drain any remaining out-proj work for qc-1, start qc's
            if gout is not None:
                for _ in gout:
                    pass
            gout = gen_outproj(qc)
        for _ in gout:
            pass


def build_nc(d, rep=1):
    nc = bacc.Bacc(None)
    # ISA reports ~224KB/partition active SBUF but only ~208KB is usable on
    # this part; allocating above that wedges the core (observed on HW).
    nc.sbuf_top = min(nc.sbuf_top, 208 * 1024)
    xT_h = nc.dram_tensor("xT", [d.C, d.S], BF16, kind="ExternalInput")
    wqkT_h = nc.dram_tensor("wqkT", [d.C, 2 * d.CH], BF16, kind="ExternalInput")
    wvT_h = nc.dram_tensor("wvT", [d.C, d.CH], BF16, kind="ExternalInput")
    wpT_h = nc.dram_tensor("wpT", [d.CH, d.C], BF16, kind="ExternalInput")
    bqk_h = nc.dram_tensor("bqk", [2 * d.CH], F32, kind="ExternalInput")
    bv_h = nc.dram_tensor("bv", [d.CH], F32, kind="ExternalInput")
    maskT_h = nc.dram_tensor("maskT", [d.S, d.S], BF16, kind="ExternalInput")
    y_h = nc.dram_tensor("y", [d.S, d.C], F32, kind="ExternalOutput")
    io = (xT_h, wqkT_h, wvT_h, wpT_h, bqk_h, bv_h, maskT_h, y_h)
    with tile.TileContext(nc) as tc:
        with ExitStack() as ctx:
            emit_body(ctx, tc, d, io, rep=rep)
    nc.compile()
    return nc


def to_bf16(a):
    import ml_dtypes
    return np.asarray(a, np.float32).astype(ml_dtypes.bfloat16)


def to_fp8(a):
    return np.asarray(a, np.float32).astype(mybir.dt.np(FP8))


def host_prep_batch(d, x_b, mask_b):
    """x_b [S, C] f32, mask_b [S, S] bool (True = masked out)."""
    xT = to_bf16(np.ascontiguousarray(x_b.T))
    maskT = to_bf16(np.ascontiguousarray((~mask_b).T))  # keep-mask, [k, q]
    return xT, maskT


SW_HOST = np.float32(SW)


def host_prep_half(d, w_qkv, b_qkv, w_proj, hh):
    C, CH = d.C, d.CH
    scale = np.float32(d.HD ** -0.5)
    r0, r1 = hh * CH, (hh + 1) * CH
    wq = w_qkv[r0:r1] * scale
    wk = w_qkv[C + r0:C + r1]
    wv = w_qkv[2 * C + r0:2 * C + r1]
    wqkT = to_bf16(np.ascontiguousarray(
        np.concatenate([wq, wk], axis=0).T, dtype=np.float32))
    wvT = to_bf16(np.ascontiguousarray(wv.T, dtype=np.float32))
    wpT = to_bf16(np.ascontiguousarray(w_proj[:, r0:r1].T, dtype=np.float32))
    bqk = np.concatenate([b_qkv[r0:r1] * scale, b_qkv[C + r0:C + r1]]).astype(np.float32)
    bv = b_qkv[2 * C + r0:2 * C + r1].astype(np.float32)
    return wqkT, wvT, wpT, bqk, bv


_NC_CACHE = {}


def kernel(x, w_qkv, b_qkv, w_proj, b_proj, attn_mask):
    from concourse.bass_utils import run_bass_kernel_spmd
    d = FULL
    B = x.shape[0]
    x = np.asarray(x, dtype=np.float32)
    attn_mask = np.asarray(attn_mask)
    w_qkv = np.asarray(w_qkv, np.float32)
    b_qkv = np.asarray(b_qkv, np.float32)
    w_proj = np.asarray(w_proj, np.float32)
    b_proj = np.asarray(b_proj, np.float32)
    halves = [host_prep_half(d, w_qkv, b_qkv, w_proj, hh) for hh in range(2)]
    in_maps = []
    for c in range(8):
        b, hh = c // 2, c % 2
        xT, maskT = host_prep_batch(d, x[b], np.asarray(attn_mask[b, 0], bool))
        wqkT, wvT, wpT, bqk, bv = halves[hh]
        in_maps.append(dict(xT=xT, wqkT=wqkT, wvT=wvT, wpT=wpT, bqk=bqk, bv=bv,
                            maskT=maskT))
    if "nc" not in _NC_CACHE:
        _NC_CACHE["nc"] = build_nc(d, rep=1)
    nc = _NC_CACHE["nc"]
    res = run_bass_kernel_spmd(nc, in_maps, core_ids=list(range(8)))
    out = np.empty((B, d.S, d.C), np.float32)
    for b in range(B):
        out[b] = (np.asarray(res.results[2 * b]["y"], np.float32)
                  + np.asarray(res.results[2 * b + 1]["y"], np.float32)
                  + b_proj[None, :].astype(np.float32))
    return out
